# revision 1
# baseline (speedup 1.0000x reference)
"""Trainium2 Bass kernel for DifferentiableVietorisRips.

Output M = concat([eye(N); pair-masks; triple-masks]) with
  N = 128, D = 512, EPSILON = 32.0, SHARPNESS = 10.0, VR_DIM = 2
  pair rows   : P = C(128,2) = 8128,  row(i,j) has sigmoid(10*(32-d_ij)) at cols i,j
  triple rows : T = C(128,3) = 341376, row(i,j,k) has [d_ij<=32 & d_jk<=32 & d_ik<=32]
                at cols i,j,k
  M shape [349632, 128] float32.

Sharding: contiguous row chunks, 43704 rows/core across 8 cores. One uniform
SPMD Bass program; every per-core difference lives in input tensors.

Per-core device pipeline:
  1. dist [128,128] via PE: d2 = sum_k(-2 W^T)^T W^T + (sq x 1 + 1 x sq), then
     ACT sqrt(max(0, d2)).
  2. DISTREP [128, 16384]: dist flattened + replicated on every partition
     (PE K=1 ones-broadcast matmuls + ACT PSUM->SBUF copies).
  3. GPSIMD indirect_copy gathers d(i,j), d(j,k), d(i,k) per output row
     (indices static per core, 16x group-redundant), ACT strided compaction,
     DVE compares/products -> c in [0,1], folded into comparand indices:
     k' = (k+1)*c_eff - 1  (c_eff=0 -> -1 -> matches nothing).
  4. Per 128-row block: mask = (IOTA==k') + (IOTA==j') + (IOTA==i') -- three
     DVE ops in bf16. Pair/m0 blocks additionally ACT-scale by per-row value
     (sigmoid for pairs, 1.0 otherwise).
  5. bf16 supertiles -> SWDGE cast-DMA -> f32 DRAM output shards (~1MB each).
"""

import math
from contextlib import ExitStack

import numpy as np
import ml_dtypes

import concourse.bass as bass
import concourse.bacc as bacc
import concourse.tile as tile
from concourse import library_config, mybir
from concourse.bass_utils import run_bass_kernel_spmd
from concourse.tile_rust import add_dep_helper

# ---------------------------------------------------------------- constants
N = 128
D = 512
EPS = 32.0
SHARP = 10.0
NCORES = 8

P_PAIRS = N * (N - 1) // 2            # 8128
T_TRIS = N * (N - 1) * (N - 2) // 6   # 341376
R_TOT = N + P_PAIRS + T_TRIS          # 349632
RC = R_TOT // NCORES                  # 43704 rows per core
NB = (RC + 127) // 128                # 342 blocks per core (last has 56 rows)
TAIL = RC - (NB - 1) * 128            # 56
NBV = 65                              # blocks 0..64 get the per-row value scale
SUPER = 16                            # blocks per output DMA

_DT = mybir.dt


# ---------------------------------------------------------------- host tables
def _host_tables():
    """Static per-core tensors (independent of W)."""
    iu, ju = np.triu_indices(N, k=1)                      # pair lex order
    # triples in lex order
    ti, tj, tk = [], [], []
    for i in range(N - 2):
        for j in range(i + 1, N - 1):
            ks = np.arange(j + 1, N)
            ti.append(np.full(len(ks), i))
            tj.append(np.full(len(ks), j))
            tk.append(ks)
    ti = np.concatenate(ti).astype(np.int64)
    tj = np.concatenate(tj).astype(np.int64)
    tk = np.concatenate(tk).astype(np.int64)
    assert ti.shape[0] == T_TRIS

    # global row r -> comparand/base tables
    k1 = np.zeros(R_TOT, np.float32)   # TK1 = (col_k)+1, 0 if unused
    j1 = np.zeros(R_TOT, np.float32)
    i1 = np.zeros(R_TOT, np.float32)
    cm = np.zeros(R_TOT, np.float32)   # 1 iff triple row (fold c)
    vsel = np.zeros(R_TOT, np.float32)  # 1 iff pair row (scale by sigmoid)
    ix1 = np.zeros(R_TOT, np.int16)   # flat idx into dist for d(i,j)
    ix2 = np.zeros(R_TOT, np.int16)   # d(j,k)
    ix3 = np.zeros(R_TOT, np.int16)   # d(i,k)
    ixp = np.zeros(R_TOT, np.int16)   # pair d(i,j) for sigmoid

    r = np.arange(N)
    k1[:N] = r + 1.0                                      # m0: single col i

    s = N
    k1[s:s + P_PAIRS] = ju + 1.0
    j1[s:s + P_PAIRS] = iu + 1.0
    vsel[s:s + P_PAIRS] = 1.0
    ixp[s:s + P_PAIRS] = (iu * N + ju).astype(np.int16)

    s = N + P_PAIRS
    k1[s:] = tk + 1.0
    j1[s:] = tj + 1.0
    i1[s:] = ti + 1.0
    cm[s:] = 1.0
    ix1[s:] = (ti * N + tj).astype(np.int16)
    ix2[s:] = (tj * N + tk).astype(np.int16)
    ix3[s:] = (ti * N + tk).astype(np.int16)

    def shard(a, core, nb, pad_shape):
        """rows [core*RC, core*RC+128*nb) -> [128, nb] (p, b), zero padded."""
        lo = core * RC
        take = min(RC, 128 * nb, a.shape[0] - lo)
        full = np.zeros(128 * nb, a.dtype)
        full[:take] = a[lo:lo + take]
        return full.reshape(nb, 128).T.copy()

    per_core = []
    for c in range(NCORES):
        t = {
            "TK1": shard(k1, c, NB, None),
            "TJ1": shard(j1, c, NB, None),
            "TI1": shard(i1, c, NB, None),
            "CM": shard(cm, c, NB, None),
            "VSEL": shard(vsel, c, NBV, None),
            "IX1": shard(ix1, c, NB, None),
            "IX2": shard(ix2, c, NB, None),
            "IX3": shard(ix3, c, NB, None),
            "IXP": shard(ixp, c, NBV, None),
        }
        t["ICM"] = (1.0 - t["CM"]).astype(np.float32)
        t["IVSEL"] = (1.0 - t["VSEL"]).astype(np.float32)
        per_core.append(t)

    iota = np.tile(np.arange(128, dtype=np.float32), (128, 1)).astype(
        ml_dtypes.bfloat16
    )
    ident = np.eye(128, dtype=np.float32)
    # residue mask for gather compaction: m16[p, 16*b + r] = (p % 16 == r)
    rmod = (np.arange(128) % 16)[:, None]
    rr = np.tile(np.arange(16), NB)[None, :]
    m16 = (rmod == rr).astype(ml_dtypes.bfloat16)
    return per_core, iota, ident, m16


# ---------------------------------------------------------------- bass program
def _build_program():
    # Bacc (not raw Bass): lowers Tile's multi-wait drain/barrier sync into
    # walrus-encodable form and auto-inserts modify_pool_config for
    # load_library. detect_race_conditions=False: the sim's race shadow
    # mis-models some APs; ordering is via Tile deps + add_dep_helper edges.
    nc = bacc.Bacc(
        "TRN2", target_bir_lowering=False, debug=False,
        detect_race_conditions=False,
    )

    f32, bf16, u16 = _DT.float32, _DT.bfloat16, _DT.int16
    W_p = nc.declare_dram_parameter("W", [N, D], f32, isOutput=False)
    IOTA_p = nc.declare_dram_parameter("IOTA", [128, 128], bf16, isOutput=False)
    IDENT_p = nc.declare_dram_parameter("IDENT", [128, 128], f32, isOutput=False)
    TK1_p = nc.declare_dram_parameter("TK1", [128, NB], f32, isOutput=False)
    TJ1_p = nc.declare_dram_parameter("TJ1", [128, NB], f32, isOutput=False)
    TI1_p = nc.declare_dram_parameter("TI1", [128, NB], f32, isOutput=False)
    CM_p = nc.declare_dram_parameter("CM", [128, NB], f32, isOutput=False)
    ICM_p = nc.declare_dram_parameter("ICM", [128, NB], f32, isOutput=False)
    VSEL_p = nc.declare_dram_parameter("VSEL", [128, NBV], f32, isOutput=False)
    IVSEL_p = nc.declare_dram_parameter("IVSEL", [128, NBV], f32, isOutput=False)
    IX1_p = nc.declare_dram_parameter("IX1", [128, NB], u16, isOutput=False)
    IX2_p = nc.declare_dram_parameter("IX2", [128, NB], u16, isOutput=False)
    IX3_p = nc.declare_dram_parameter("IX3", [128, NB], u16, isOutput=False)
    IXP_p = nc.declare_dram_parameter("IXP", [128, NBV], u16, isOutput=False)
    M16_p = nc.declare_dram_parameter("M16", [128, 16 * NB], bf16, isOutput=False)
    OUT_p = nc.declare_dram_parameter("out", [RC, 128], f32, isOutput=True)

    with tile.TileContext(nc) as tc, ExitStack() as ctx:
        const = ctx.enter_context(tc.tile_pool(name="const", bufs=1))
        work = ctx.enter_context(tc.tile_pool(name="work", bufs=1))
        psum = ctx.enter_context(tc.tile_pool(name="psum", bufs=1, space="PSUM"))
        psum2 = ctx.enter_context(tc.tile_pool(name="psum2", bufs=2, space="PSUM"))
        gpool = ctx.enter_context(tc.tile_pool(name="gath", bufs=1))
        tmp = ctx.enter_context(tc.tile_pool(name="tmp", bufs=6))
        sup = ctx.enter_context(tc.tile_pool(name="sup", bufs=3))

        # first Pool-engine instruction: select the ucode library that
        # implements InstAPGather (the only custom gpsimd op we use)
        nc.gpsimd.load_library(library_config.ap_gather)

        load_instrs = {}

        def load(pool, param, shape, dt):
            t = pool.tile(shape, dt, tag=param.name)
            load_instrs[param.name] = nc.sync.dma_start(t[:], param.ap())
            return t

        w_sb = load(const, W_p, [N, D], f32)
        iota = load(const, IOTA_p, [128, 128], bf16)
        ident = load(const, IDENT_p, [128, 128], f32)
        tk1 = load(const, TK1_p, [128, NB], f32)
        tj1 = load(const, TJ1_p, [128, NB], f32)
        ti1 = load(const, TI1_p, [128, NB], f32)
        cmt = load(const, CM_p, [128, NB], f32)
        icmt = load(const, ICM_p, [128, NB], f32)
        vselt = load(const, VSEL_p, [128, NBV], f32)
        ivselt = load(const, IVSEL_p, [128, NBV], f32)
        ix1 = load(const, IX1_p, [128, NB], u16)
        ix2 = load(const, IX2_p, [128, NB], u16)
        ix3 = load(const, IX3_p, [128, NB], u16)
        ixp = load(const, IXP_p, [128, NBV], u16)
        m16rep = load(const, M16_p, [128, 16 * NB], bf16)

        # ---- 1. dist --------------------------------------------------------
        ww = work.tile([N, D], f32)
        nc.vector.tensor_tensor(ww[:], w_sb[:], w_sb[:], mybir.AluOpType.mult)
        sq = work.tile([N, 1], f32)
        nc.vector.tensor_reduce(
            sq[:], ww[:], mybir.AxisListType.X, mybir.AluOpType.add
        )

        wts, wtm2s = [], []
        for c4 in range(4):
            pst = psum2.tile([128, 128], f32, tag="ptrans")
            nc.tensor.transpose(pst[:], w_sb[:, c4 * 128:(c4 + 1) * 128], ident[:])
            wt = work.tile([128, 128], f32, tag=f"wt{c4}")
            nc.vector.tensor_copy(wt[:], pst[:])
            wtm2 = work.tile([128, 128], f32, tag=f"wtm2{c4}")
            nc.vector.tensor_scalar_mul(wtm2[:], pst[:], -2.0)
            wts.append(wt)
            wtm2s.append(wtm2)

        # aug_l = [sq_row; ones], aug_r = [ones; sq_row] via PE transpose of
        # [128, 2] column pairs (engines can't write at partition offset 1)
        cat_l = work.tile([128, 2], f32)
        nc.vector.tensor_copy(cat_l[:, 0:1], sq[:])
        nc.vector.memset(cat_l[:, 1:2], 1.0)
        cat_r = work.tile([128, 2], f32)
        nc.vector.memset(cat_r[:, 0:1], 1.0)
        nc.vector.tensor_copy(cat_r[:, 1:2], sq[:])
        paug_l = psum2.tile([2, 128], f32, tag="paug")
        nc.tensor.transpose(paug_l[:], cat_l[:], ident[:])
        aug_l = work.tile([2, 128], f32)
        nc.vector.tensor_copy(aug_l[:], paug_l[:])
        paug_r = psum2.tile([2, 128], f32, tag="paug")
        nc.tensor.transpose(paug_r[:], cat_r[:], ident[:])
        aug_r = work.tile([2, 128], f32)
        nc.vector.tensor_copy(aug_r[:], paug_r[:])

        d2 = psum.tile([128, 128], f32, tag="d2")
        for c4 in range(4):
            nc.tensor.matmul(
                d2[:], wtm2s[c4][:], wts[c4][:], start=(c4 == 0), stop=False
            )
        nc.tensor.matmul(d2[:], aug_l[:], aug_r[:], start=False, stop=True)

        dmax = work.tile([128, 128], f32)
        nc.vector.tensor_scalar_max(dmax[:], d2[:], 0.0)
        dist = work.tile([128, 128], f32)
        nc.scalar.activation(dist[:], dmax[:], mybir.ActivationFunctionType.Sqrt)

        # ---- 2. packed table: BV = 2*(dist<=eps) + sigmoid(10*(eps-dist)) ---
        # one f32 table serves both the triple condition (BV >= 1.5) and the
        # pair value (BV - 2*(BV >= 1.5)); sigmoid in (0,1) keeps the bands
        # [0,1) and [2,3) cleanly separable.
        bind = work.tile([128, 128], f32)
        nc.vector.tensor_scalar(bind[:], dist[:], EPS, None, mybir.AluOpType.is_le)
        sigb = work.tile([128, 1], f32)
        nc.vector.memset(sigb[:], SHARP * EPS)
        sgm = work.tile([128, 128], f32)
        nc.scalar.activation(
            sgm[:], dist[:], mybir.ActivationFunctionType.Sigmoid,
            bias=sigb[:], scale=-SHARP,
        )
        bv = work.tile([128, 128], f32)
        nc.vector.scalar_tensor_tensor(
            bv[:], bind[:], 2.0, sgm[:],
            mybir.AluOpType.mult, mybir.AluOpType.add,
        )

        # flatten into partition 0 of the table, then broadcast via PE
        # ones-outer-product (all APs based at partition 0), 512-col chunks
        ones_row = work.tile([1, 128], f32)
        nc.vector.memset(ones_row[:], 1.0)
        bvtab = work.tile([128, 128 * 128], f32)
        nc.sync.dma_start(bvtab[0:1, :], bv[:])
        rep_writers = []
        for ch in range(32):
            sl = slice(ch * 512, (ch + 1) * 512)
            pbc = psum2.tile([128, 512], f32, tag="pbc")
            nc.tensor.matmul(
                pbc[:], ones_row[:], bvtab[0:1, sl], start=True, stop=True
            )
            if ch % 2 == 0:
                rep_writers.append(nc.scalar.copy(bvtab[:, sl], pbc[:]))
            else:
                rep_writers.append(nc.vector.tensor_copy(bvtab[:, sl], pbc[:]))

        # ---- 3. gathers -> c -> folded comparands ---------------------------
        # ap_gather uses group-shared indices: 16 partitions of a Q7 core read
        # the same flat index, so each value lands 16x redundant. Slot y=16b+r
        # of the output serves partition-class r for block b; compaction is
        # (multiply by the static residue mask) then (reduce over r).
        # ap_gather's for_isa APs are invisible to Tile's dep tracker; wire
        # ordering explicitly.
        prev_use = []

        def gather(ixt, ixt_name, nb):
            nonlocal prev_use
            g = gpool.tile([128, 16 * nb], f32, tag="gbuf")
            gi = nc.gpsimd.ap_gather(
                g[:], bvtab[:], ixt[:],
                channels=128, num_elems=128 * 128, d=1, num_idxs=16 * nb,
            )
            for w in rep_writers:
                add_dep_helper(gi.ins, w.ins, reason="gather after table")
            add_dep_helper(
                gi.ins, load_instrs[ixt_name].ins, reason="gather after idx load"
            )
            for pu in prev_use:
                add_dep_helper(gi.ins, pu.ins, reason="gbuf reuse WAR")
            prev_use = []
            return g, gi

        def compact(red_bf16, nb, name):
            """[128, 16*nb] (bf16, zero except own-class slots) -> [128, nb]."""
            cx = work.tile([128, nb], f32, tag=name)
            nc.vector.tensor_reduce(
                cx[:],
                red_bf16[:].rearrange("p (b r) -> p b r", r=16),
                mybir.AxisListType.X,
                mybir.AluOpType.add,
            )
            return cx

        # triple condition: decode each gather to {0,1} bf16, product, mask
        bred = []
        for ixt, ixn in ((ix1, "IX1"), (ix2, "IX2"), (ix3, "IX3")):
            g, gi = gather(ixt, ixn, NB)
            bt = gpool.tile([128, 16 * NB], bf16, tag=f"bred{len(bred)}")
            di = nc.vector.tensor_scalar(
                bt[:], g[:], 1.5, None, mybir.AluOpType.is_ge
            )
            add_dep_helper(di.ins, gi.ins, reason="decode after gather")
            prev_use = [di]
            bred.append(bt)
        nc.vector.tensor_tensor(
            bred[0][:], bred[0][:], bred[1][:], mybir.AluOpType.mult
        )
        nc.vector.tensor_tensor(
            bred[0][:], bred[0][:], bred[2][:], mybir.AluOpType.mult
        )
        nc.vector.tensor_tensor(
            bred[0][:], bred[0][:], m16rep[:], mybir.AluOpType.mult
        )
        cc = compact(bred[0], NB, "cc")

        # pair value: sigma = BV - 2*(BV>=1.5), masked then compacted
        gp, gpi = gather(ixp, "IXP", NBV)
        ip = gpool.tile([128, 16 * NBV], f32, tag="ipair")
        di = nc.vector.tensor_scalar(ip[:], gp[:], 1.5, None, mybir.AluOpType.is_ge)
        add_dep_helper(di.ins, gpi.ins, reason="decode after gather")
        prev_use = [di]
        sp = nc.vector.scalar_tensor_tensor(
            ip[:], ip[:], -2.0, gp[:], mybir.AluOpType.mult, mybir.AluOpType.add
        )
        prev_use = [sp]
        nc.vector.tensor_tensor(
            ip[:], ip[:], m16rep[:, 0:16 * NBV], mybir.AluOpType.mult
        )
        dpair = compact(ip, NBV, "dpair")

        ce1 = work.tile([128, NB], f32)
        nc.vector.tensor_tensor(ce1[:], cc[:], cmt[:], mybir.AluOpType.mult)
        ceff = work.tile([128, NB], f32)
        nc.vector.tensor_tensor(ceff[:], ce1[:], icmt[:], mybir.AluOpType.add)

        def fold(base, name):
            t = work.tile([128, NB], f32, tag=name + "_f")
            nc.vector.tensor_tensor(t[:], base[:], ceff[:], mybir.AluOpType.mult)
            p = work.tile([128, NB], f32, tag=name)
            nc.vector.tensor_scalar(
                p[:], t[:], 1.0, None, mybir.AluOpType.subtract
            )
            return p

        tkp = fold(tk1, "tkp")
        tjp = fold(tj1, "tjp")
        tip = fold(ti1, "tip")

        # per-row scale for blocks 0..64: dpair already holds the sigmoid
        # values (decoded from the packed table); 1.0 on non-pair rows
        vtmp = work.tile([128, NBV], f32)
        nc.vector.tensor_tensor(vtmp[:], dpair[:], vselt[:], mybir.AluOpType.mult)
        vs = work.tile([128, NBV], f32)
        nc.vector.tensor_tensor(vs[:], vtmp[:], ivselt[:], mybir.AluOpType.add)

        # ---- 4. block chain + output DMAs -----------------------------------
        eq, add_ = mybir.AluOpType.is_equal, mybir.AluOpType.add
        n_sup = (NB + SUPER - 1) // SUPER  # 22
        for s in range(n_sup):
            b_lo = s * SUPER
            b_hi = min(NB, b_lo + SUPER)
            nblk = b_hi - b_lo
            st = sup.tile([128, SUPER * 128], bf16, tag="super")
            for b in range(b_lo, b_hi):
                sl = slice((b - b_lo) * 128, (b - b_lo + 1) * 128)
                t1 = tmp.tile([128, 128], bf16, tag="t1")
                nc.vector.tensor_scalar(t1[:], iota[:], tkp[:, b:b + 1], None, eq)
                if b < NBV:
                    t2 = tmp.tile([128, 128], bf16, tag="t2")
                    nc.vector.scalar_tensor_tensor(
                        t2[:], iota[:], tjp[:, b:b + 1], t1[:], eq, add_
                    )
                    t3 = tmp.tile([128, 128], bf16, tag="t3")
                    nc.vector.scalar_tensor_tensor(
                        t3[:], iota[:], tip[:, b:b + 1], t2[:], eq, add_
                    )
                    nc.scalar.activation(
                        st[:, sl], t3[:], mybir.ActivationFunctionType.Copy,
                        bias=0.0, scale=vs[:, b:b + 1],
                    )
                else:
                    t2 = tmp.tile([128, 128], bf16, tag="t2")
                    nc.vector.scalar_tensor_tensor(
                        t2[:], iota[:], tjp[:, b:b + 1], t1[:], eq, add_
                    )
                    nc.vector.scalar_tensor_tensor(
                        st[:, sl], iota[:], tip[:, b:b + 1], t2[:], eq, add_
                    )
            # DMA out per block slot (SWDGE casts bf16 -> f32); 64KB each,
            # fixed costs overlap across the deep pipeline
            for b in range(b_lo, b_hi):
                sl = slice((b - b_lo) * 128, (b - b_lo + 1) * 128)
                rows = 128 if b < NB - 1 else TAIL
                dst = OUT_p.ap()[b * 128:b * 128 + rows, :]
                nc.gpsimd.dma_start(dst, st[0:rows, sl])

    nc.compile()
    return nc


_PROGRAM = None
_TABLES = None


def _get_program():
    global _PROGRAM, _TABLES
    if _PROGRAM is None:
        _TABLES = _host_tables()
        _PROGRAM = _build_program()
    return _PROGRAM, _TABLES


def kernel(W: np.ndarray) -> np.ndarray:
    nc, (per_core, iota, ident, m16) = _get_program()
    W = np.ascontiguousarray(np.asarray(W, dtype=np.float32))
    in_maps = []
    for c in range(NCORES):
        t = per_core[c]
        in_maps.append({
            "W": W,
            "IOTA": iota,
            "IDENT": ident,
            "TK1": t["TK1"], "TJ1": t["TJ1"], "TI1": t["TI1"],
            "CM": t["CM"], "ICM": t["ICM"],
            "VSEL": t["VSEL"], "IVSEL": t["IVSEL"],
            "IX1": t["IX1"], "IX2": t["IX2"], "IX3": t["IX3"],
            "IXP": t["IXP"], "M16": m16,
        })
    res = run_bass_kernel_spmd(nc, in_maps, list(range(NCORES)))
    shards = [np.asarray(res.results[c]["out"]) for c in range(NCORES)]
    return np.concatenate(shards, axis=0).astype(np.float32)

